# revision 19
# baseline (speedup 1.0000x reference)
"""KNN graph kernel for Trainium2 (8 NeuronCores, SPMD).

Problem: x [16384, 128] f32 -> indices of the 16 nearest neighbors per row
(excluding self) by Euclidean distance, [16384, 16] int32.

Math: rank ascending d2[i,j] == rank descending s[i,j] = G[i,j] - 0.5*sq[j]
(G = x@x.T, sq[j] = ||x_j||^2; the row-constant sq[i]/2 and the monotone sqrt
do not change per-row ranking).  Per-row top-17 largest s, drop rank 0 (self).

Sharding: rows split across 8 cores (2048 rows each); every core holds the
full x^T (replicated) for the right-hand side.

Per core (v3 — hi/lo bf16 matmuls, 1024-wide chunks, stage C on Pool):
  - PE: S tile [128, 1024] = 3 bf16 matmuls per 512 half (h.h + h.l + l.h,
        x = h + l split on host; l.l term ~2^-18 dropped)
        + ones2.T @ negsqhalf hi/lo (fp16, K=2)
  - ACT: evict PSUM -> SBUF f32
  - DVE: per-1024-chunk top-8 (max + max_index), 16 chunks -> 128 candidates
        stage B: 3 rounds max8/match_replace on the 128-wide candidate row
  - Pool: candidate global-index arithmetic + stage C one-hot extraction
  - DMA out [128, 16] int32 per row block.
"""
import numpy as np

N = 16384
D = 128
KOUT = 16
NCORES = 8
ROWS_PER_CORE = N // NCORES          # 2048
RB = ROWS_PER_CORE // 128            # 16 row blocks per core
CHUNK = 1024                         # scan chunk width
NCHUNK = N // CHUNK                  # 16
CANDW = NCHUNK * 8                   # 128 candidates per row
SETCH = 512                          # setup chunk width (1 PSUM bank)

_nc_cache = None


def build_nc():
    import concourse.bass as bass
    import concourse.bacc as bacc
    import concourse.mybir as mybir
    import concourse.tile as tile

    f32 = mybir.dt.float32
    bf16 = mybir.dt.bfloat16
    f16 = mybir.dt.float16
    i32 = mybir.dt.int32
    u16 = mybir.dt.uint16

    nc = bacc.Bacc("TRN2", target_bir_lowering=False, debug=False)
    xth = nc.dram_tensor("xth", [D, N], bf16, kind="ExternalInput")
    xtl = nc.dram_tensor("xtl", [D, N], bf16, kind="ExternalInput")
    xtlh = nc.dram_tensor("xtlh", [D, ROWS_PER_CORE], bf16, kind="ExternalInput")
    xtll = nc.dram_tensor("xtll", [D, ROWS_PER_CORE], bf16, kind="ExternalInput")
    nsq = nc.dram_tensor("nsq", [2, N], f16, kind="ExternalInput")
    out = nc.dram_tensor("out", [ROWS_PER_CORE, KOUT], i32, kind="ExternalOutput")

    with tile.TileContext(nc) as tc:
        with tc.tile_pool(name="persist", bufs=1) as persist, \
             tc.tile_pool(name="psum", bufs=4, space="PSUM") as psum, \
             tc.tile_pool(name="sbuf", bufs=4) as sbuf, \
             tc.tile_pool(name="cand", bufs=2) as cand, \
             tc.tile_pool(name="small", bufs=2) as small:

            # ---- load inputs (xt split so consumers unblock progressively) ----
            xth_sb = persist.tile([D, N], bf16)
            xtl_sb = persist.tile([D, N], bf16)
            xtlh_sb = persist.tile([D, ROWS_PER_CORE], bf16)
            xtll_sb = persist.tile([D, ROWS_PER_CORE], bf16)
            negsq2 = persist.tile([2, N], f16)     # row 0 = hi, row 1 = lo
            # chunk-granular, interleaved in consumption order: rb0/chunk c
            # needs xtloc h+l [:, :128], xt h+l [:, c*1K:(c+1)*1K], negsq2.
            nc.sync.dma_start(xtlh_sb[:, 0:128], xtlh.ap()[:, 0:128])
            nc.sync.dma_start(xtll_sb[:, 0:128], xtll.ap()[:, 0:128])
            for c in range(NCHUNK):
                sl = slice(c * CHUNK, (c + 1) * CHUNK)
                nc.sync.dma_start(xth_sb[:, sl], xth.ap()[:, sl])
                nc.sync.dma_start(xtl_sb[:, sl], xtl.ap()[:, sl])
                nc.sync.dma_start(negsq2[:, sl], nsq.ap()[:, sl])
            nc.sync.dma_start(xtlh_sb[:, 128:], xtlh.ap()[:, 128:])
            nc.sync.dma_start(xtll_sb[:, 128:], xtll.ap()[:, 128:])

            # ---- constants ----
            ones2 = persist.tile([2, 128], f16)
            nc.vector.memset(ones2[:], 1.0)
            # lutbase[p, slot] = (slot // 8) * CHUNK, same on all partitions
            lut_i = persist.tile([128, CANDW], i32)
            nc.gpsimd.iota(lut_i[:].rearrange("p (c k) -> p c k", k=8),
                           pattern=[[CHUNK, NCHUNK], [0, 8]], base=0,
                           channel_multiplier=0)
            lutbase = persist.tile([128, CANDW], f32)
            nc.gpsimd.tensor_copy(lutbase[:], lut_i[:])
            # iota128[p, slot] = slot
            iota_i = persist.tile([128, CANDW], i32)
            nc.gpsimd.iota(iota_i[:], pattern=[[1, CANDW]], base=0,
                           channel_multiplier=0)
            iota128 = persist.tile([128, CANDW], f32)
            nc.gpsimd.tensor_copy(iota128[:], iota_i[:])

            # ---- main loop ----
            for rb in range(RB):
                rsl = slice(rb * 128, (rb + 1) * 128)
                lhs_h = xtlh_sb[:, rsl]
                lhs_l = xtll_sb[:, rsl]
                candV = cand.tile([128, CANDW], f32, tag="candV")
                candI = cand.tile([128, CANDW], u16, tag="candI")
                for c in range(NCHUNK):
                    ps = psum.tile([128, CHUNK], f32, tag="mm")
                    for h in range(2):
                        sl = slice(c * CHUNK + h * 512, c * CHUNK + (h + 1) * 512)
                        psl = ps[:, h * 512:(h + 1) * 512]
                        nc.tensor.matmul(psl, lhs_h, xth_sb[:, sl],
                                         start=True, stop=False)
                        nc.tensor.matmul(psl, lhs_h, xtl_sb[:, sl],
                                         start=False, stop=False)
                        nc.tensor.matmul(psl, lhs_l, xth_sb[:, sl],
                                         start=False, stop=False)
                        nc.tensor.matmul(psl, ones2[:], negsq2[:, sl],
                                         start=False, stop=True)
                    s_sb = sbuf.tile([128, CHUNK], f32, tag="s")
                    nc.scalar.copy(s_sb[:], ps[:])
                    nc.vector.max(candV[:, c * 8:(c + 1) * 8], s_sb[:])
                    nc.vector.max_index(candI[:, c * 8:(c + 1) * 8],
                                        candV[:, c * 8:(c + 1) * 8], s_sb[:])

                # global candidate indices = candI + (slot//8)*CHUNK  (Pool)
                candIG = cand.tile([128, CANDW], f32, tag="candIG")
                nc.gpsimd.tensor_copy(candIG[:], candI[:])
                nc.gpsimd.tensor_add(candIG[:], candIG[:], lutbase[:])

                # stage B: top-17 of candV with positions (DVE)
                v8a = small.tile([128, 8], f32, tag="v8a")
                v8b = small.tile([128, 8], f32, tag="v8b")
                v8c = small.tile([128, 8], f32, tag="v8c")
                posf = small.tile([128, 24], f32, tag="posf")
                pos_u = small.tile([128, 24], u16, tag="posu")
                candV2 = cand.tile([128, CANDW], f32, tag="candV2")
                candV3 = cand.tile([128, CANDW], f32, tag="candV3")

                nc.vector.max(v8a[:], candV[:])
                nc.vector.max_index(pos_u[:, 0:8], v8a[:], candV[:])
                nc.vector.match_replace(candV2[:], v8a[:], candV[:], -3.0e38)
                nc.vector.max(v8b[:], candV2[:])
                nc.vector.max_index(pos_u[:, 8:16], v8b[:], candV2[:])
                nc.vector.match_replace(candV3[:], v8b[:], candV2[:], -3.0e38)
                nc.vector.max(v8c[:], candV3[:])
                nc.vector.max_index(pos_u[:, 16:24], v8c[:], candV3[:])

                nc.gpsimd.tensor_copy(posf[:], pos_u[:])

                # stage C (DVE; TensorScalarPtr is not HW-legal on Pool):
                # winIG[p, k-1] = candIG[p, pos[p, k]], k=1..16
                winIG = small.tile([128, KOUT], f32, tag="winIG")
                for k in range(1, KOUT + 1):
                    scratch = cand.tile([128, CANDW], f32, tag="scratch")
                    nc.vector.scalar_tensor_tensor(
                        scratch[:], iota128[:], posf[:, k:k + 1], candIG[:],
                        op0=mybir.AluOpType.is_equal,
                        op1=mybir.AluOpType.mult,
                        accum_out=winIG[:, k - 1:k])

                out_i = small.tile([128, KOUT], i32, tag="outi")
                nc.gpsimd.tensor_copy(out_i[:], winIG[:])
                nc.sync.dma_start(out.ap()[rb * 128:(rb + 1) * 128, :], out_i[:])

    nc.compile()
    return nc


LAST_RESULT = None


def kernel(inputs: np.ndarray) -> np.ndarray:
    from concourse.bass_utils import run_bass_kernel_spmd

    global _nc_cache, LAST_RESULT
    if _nc_cache is None:
        _nc_cache = build_nc()
    nc = _nc_cache

    import ml_dtypes

    bf16 = ml_dtypes.bfloat16
    x = np.asarray(inputs, dtype=np.float32)
    xt = np.ascontiguousarray(x.T)                      # [128, 16384]
    xth = xt.astype(bf16)
    xtl = (xt - xth.astype(np.float32)).astype(bf16)
    negsq = (-0.5) * (x.astype(np.float64) ** 2).sum(axis=1)
    hi = negsq.astype(np.float16)
    lo = (negsq - hi.astype(np.float64)).astype(np.float16)
    nsq2 = np.ascontiguousarray(np.stack([hi, lo]))     # [2, 16384] f16
    in_maps = []
    for c in range(NCORES):
        csl = slice(c * ROWS_PER_CORE, (c + 1) * ROWS_PER_CORE)
        in_maps.append({
            "xth": xth, "xtl": xtl,
            "xtlh": np.ascontiguousarray(xth[:, csl]),
            "xtll": np.ascontiguousarray(xtl[:, csl]),
            "nsq": nsq2,
        })
    res = run_bass_kernel_spmd(nc, in_maps, list(range(NCORES)))
    LAST_RESULT = res
    outs = [res.results[c]["out"] for c in range(NCORES)]
    return np.concatenate(outs, axis=0).astype(np.int32)
